# revision 1
# baseline (speedup 1.0000x reference)
"""HIN2vec forward kernel for 8 Trainium2 NeuronCores.

Math (per batch element b):
    agg[e]  = mean_k emb_start[neighbors[b,e,k]]          # [E, D]
    h[e]    = agg[e] @ W1 + b1                            # [E, D]
    s       = concat_e(h) @ W2 + b2                       # [D]
    v       = emb_end[end[b]] * sigmoid(emb_path[path[b]])# [D]
    out[b]  = sigmoid(s . v)

Rewritten to make the device work gather-dominated:
    s . v = sum_e agg_sum[e] . (v @ M_e^T)/16  +  c . v
      where M_e = W1 @ W2_e  (W2_e = W2[e*D:(e+1)*D]),
            c   = b2 + b1 @ (sum_e W2_e),
            agg_sum[e] = sum_k emb_start[neighbors[b,e,k]]  (no /K; folded into M).

So the 1M-row table gathers (the memory roofline) feed only a K-sum (DVE
tree adds) and one elementwise multiply + reduction; the Linear layers act
on v via tiny PE matmuls (v is 32x smaller than the gathered data).

Sharding: pure data parallel over the batch; tables + weights replicated.

Table dtype: f32 (tmode="f32"). The random 512B-row gather is row-rate
bound on the SDMA engines, not byte-bound: on HW, fp16 tables timed 171us
vs f32 198.5us (only ~14%) and fp8 176.9us, but every sub-512B-row variant
produced NaN on real hardware (CoreSim passed) — including one that casts
the fp16 gather straight to f32 before any compute, isolating the fault to
the fp16 indirect_dma_start itself — so f32 is kept. Spreading the
indirect DMAs over 2-4 SWDGE queues or deepening the gather prefetch past
2 buffers both measured slightly WORSE (172-200us at f32), hence
nqueues=1, gbufs=2.
"""

import numpy as np

B, E, K, D = 8192, 4, 16, 128
N, PPATH = 1_000_000, 64
NCORES = 8
BL = B // NCORES          # 1024 batch elements per core
NT = BL // 128            # 8 b-tiles of 128 per core
ROWS_PER_TILE = E * K     # 64 gathered rows per batch element

_CACHE = {}


def _build_nc(reps=1, nqueues=1, tmode="f32", gbufs=2):
    import concourse.bacc as bacc
    import concourse.bass as bass
    import concourse.mybir as mybir
    import concourse.tile as tile
    from concourse.masks import make_identity

    f32 = mybir.dt.float32
    i32 = mybir.dt.int32
    tdt = {"f32": f32, "f16": mybir.dt.float16, "f16c": mybir.dt.float16,
           "f8": mybir.dt.float8e4}[tmode]
    pdt = mybir.dt.float16 if tmode == "f8" else tdt
    cast16 = tmode == "f16c"  # gather fp16, cast to f32 before any compute
    AF = mybir.ActivationFunctionType

    nc = bacc.Bacc("TRN2", target_bir_lowering=False, debug=False,
                   num_swdge_queues=nqueues)

    def indirect_on_queue(qi, **kw):
        bi = nc.gpsimd.indirect_dma_start(**kw)
        if qi % nqueues:
            bi.ins.queue = f"qPoolDynamic{qi % nqueues}"
        return bi

    emb_s = nc.dram_tensor("emb_s", [N, D], tdt, kind="ExternalInput")
    emb_e = nc.dram_tensor("emb_e", [N, D], tdt, kind="ExternalInput")
    emb_p = nc.dram_tensor("emb_p", [PPATH, D], pdt, kind="ExternalInput")
    idx_nei = nc.dram_tensor("idx_nei", [128, NT * ROWS_PER_TILE], i32, kind="ExternalInput")
    idx_end = nc.dram_tensor("idx_end", [128, NT], i32, kind="ExternalInput")
    idx_path = nc.dram_tensor("idx_path", [128, NT], i32, kind="ExternalInput")
    w1_d = nc.dram_tensor("w1", [D, D], f32, kind="ExternalInput")
    b1_d = nc.dram_tensor("b1", [D, 1], f32, kind="ExternalInput")
    w2_d = nc.dram_tensor("w2", [E * D, D], f32, kind="ExternalInput")
    b2_d = nc.dram_tensor("b2", [1, D], f32, kind="ExternalInput")
    out_d = nc.dram_tensor("out", [128, NT], f32, kind="ExternalOutput")

    with tile.TileContext(nc) as tc:
        with (
            tc.tile_pool(name="const", bufs=1) as cpool,
            tc.tile_pool(name="gather", bufs=gbufs) as gpool,
            tc.tile_pool(name="tree", bufs=2) as tpool,
            tc.tile_pool(name="agg", bufs=2) as apool,
            tc.tile_pool(name="small", bufs=2) as spool,
            tc.tile_pool(name="psum_pre", bufs=1, space="PSUM") as ppre,
            tc.tile_pool(name="psum", bufs=2, space="PSUM") as pspool,
        ):
            # ---- constant loads ----
            idxn_sb = cpool.tile([128, NT * ROWS_PER_TILE], i32)
            nc.sync.dma_start(out=idxn_sb[:], in_=idx_nei[:])
            idxe_sb = cpool.tile([128, NT], i32)
            nc.sync.dma_start(out=idxe_sb[:], in_=idx_end[:])
            idxp_sb = cpool.tile([128, NT], i32)
            nc.sync.dma_start(out=idxp_sb[:], in_=idx_path[:])

            w1_sb = cpool.tile([128, D], f32)
            nc.sync.dma_start(out=w1_sb[:], in_=w1_d[:])
            w2_sb = cpool.tile([128, E * D], f32)
            for e in range(E):
                nc.sync.dma_start(
                    out=w2_sb[:, e * D:(e + 1) * D],
                    in_=w2_d[e * D:(e + 1) * D, :],
                )
            b1_sb = cpool.tile([128, 1], f32)
            nc.sync.dma_start(out=b1_sb[:], in_=b1_d[:])
            b2_sb = cpool.tile([1, D], f32)
            nc.sync.dma_start(out=b2_sb[:], in_=b2_d[:])

            ident = cpool.tile([128, 128], f32)
            make_identity(nc, ident[:])

            # ---- prelude: MT_all[f, e*D+d] = (W1^T/16 . W2_e)[f,d]^T ... i.e.
            # MT_e[f, d] = sum_x W2_e[x, f] * W1T[x, d] / 16  = (M_e/16)^T
            w1t_ps = ppre.tile([128, 128], f32)
            nc.tensor.transpose(w1t_ps[:], w1_sb[:], ident[:])
            w1t_sb = cpool.tile([128, 128], f32)
            w1t_scale = 1.0 / (K * 256.0) if tmode == "f8" else 1.0 / K
            nc.scalar.mul(w1t_sb[:], w1t_ps[:], w1t_scale)
            mt_sb = cpool.tile([128, E * D], f32)
            for e in range(E):
                mt_ps = ppre.tile([128, 128], f32, tag="mt_ps")
                nc.tensor.matmul(
                    mt_ps[:],
                    lhsT=w2_sb[:, e * D:(e + 1) * D],
                    rhs=w1t_sb[:],
                    start=True, stop=True,
                )
                nc.vector.tensor_copy(out=mt_sb[:, e * D:(e + 1) * D], in_=mt_ps[:])

            # c = b2 + b1 @ (sum_e W2_e)   -> [1, D]
            wsum_sb = cpool.tile([128, D], f32)
            nc.vector.tensor_add(out=wsum_sb[:], in0=w2_sb[:, 0:D], in1=w2_sb[:, D:2 * D])
            nc.vector.tensor_add(out=wsum_sb[:], in0=wsum_sb[:], in1=w2_sb[:, 2 * D:3 * D])
            nc.vector.tensor_add(out=wsum_sb[:], in0=wsum_sb[:], in1=w2_sb[:, 3 * D:4 * D])
            c_ps = ppre.tile([1, D], f32)
            nc.tensor.matmul(c_ps[:], lhsT=b1_sb[:], rhs=wsum_sb[:], start=True, stop=True)
            c_sb = cpool.tile([1, D], f32)
            nc.vector.tensor_add(out=c_sb[:], in0=c_ps[:], in1=b2_sb[:])
            if tmode == "f8":
                nc.scalar.mul(c_sb[:], c_sb[:], 1.0 / 16.0)
            c_bc = cpool.tile([128, D], f32)
            nc.gpsimd.partition_broadcast(c_bc[:], c_sb[:])

            # ---- v = emb_end[end] * sigmoid(emb_path[path]) : [128, NT*D] ----
            ve_sb = cpool.tile([128, NT * D], tdt)
            indirect_on_queue(1,
                out=ve_sb[:], out_offset=None,
                in_=emb_e[:],
                in_offset=bass.IndirectOffsetOnAxis(ap=idxe_sb[:], axis=0),
            )
            vp_sb = cpool.tile([128, NT * D], pdt)
            indirect_on_queue(2,
                out=vp_sb[:], out_offset=None,
                in_=emb_p[:],
                in_offset=bass.IndirectOffsetOnAxis(ap=idxp_sb[:], axis=0),
            )
            if cast16:
                ve32 = cpool.tile([128, NT * D], f32)
                nc.vector.tensor_copy(out=ve32[:], in_=ve_sb[:])
                vp32 = cpool.tile([128, NT * D], f32)
                nc.vector.tensor_copy(out=vp32[:], in_=vp_sb[:])
            else:
                ve32, vp32 = ve_sb, vp_sb
            v_sb = cpool.tile([128, NT * D], f32)
            nc.scalar.activation(v_sb[:], vp32[:], AF.Sigmoid)
            nc.vector.tensor_mul(out=v_sb[:], in0=v_sb[:], in1=ve32[:])

            res_sb = cpool.tile([128, NT], f32)

            # ---- main loop over b-tiles ----
            for t in range(NT * reps):
                t = t % NT
                g = gpool.tile([128, ROWS_PER_TILE * D], tdt, tag="g")
                indirect_on_queue(t,
                    out=g[:], out_offset=None,
                    in_=emb_s[:],
                    in_offset=bass.IndirectOffsetOnAxis(
                        ap=idxn_sb[:, t * ROWS_PER_TILE:(t + 1) * ROWS_PER_TILE],
                        axis=0,
                    ),
                )
                if cast16:
                    g32 = gpool.tile([128, ROWS_PER_TILE * D], f32, tag="g32")
                    nc.vector.tensor_copy(out=g32[:], in_=g[:])
                    g = g32

                # sum over k (tree): [128, e, k, d] -> [128, e, d]
                gv = g[:].rearrange("p (e k d) -> p e k d", e=E, k=K, d=D)
                t1 = tpool.tile([128, E * 8 * D], f32, tag="t1")
                t1v = t1[:].rearrange("p (e k d) -> p e k d", e=E, k=8, d=D)
                nc.vector.tensor_add(out=t1v, in0=gv[:, :, 0:K:2, :], in1=gv[:, :, 1:K:2, :])
                t2 = tpool.tile([128, E * 4 * D], f32, tag="t2")
                t2v = t2[:].rearrange("p (e k d) -> p e k d", e=E, k=4, d=D)
                nc.vector.tensor_add(out=t2v, in0=t1v[:, :, 0:8:2, :], in1=t1v[:, :, 1:8:2, :])
                t3 = tpool.tile([128, E * 2 * D], f32, tag="t3")
                t3v = t3[:].rearrange("p (e k d) -> p e k d", e=E, k=2, d=D)
                nc.vector.tensor_add(out=t3v, in0=t2v[:, :, 0:4:2, :], in1=t2v[:, :, 1:4:2, :])
                agg = apool.tile([128, E * D], f32, tag="agg")
                aggv = agg[:].rearrange("p (e k d) -> p e k d", e=E, k=1, d=D)
                nc.vector.tensor_add(out=aggv, in0=t3v[:, :, 0:2:2, :], in1=t3v[:, :, 1:2:2, :])

                # w[b, (e,d)] = sum_f v[b, f] * MT_all[f, (e,d)]
                vt_ps = pspool.tile([128, 128], f32, tag="vt_ps")
                nc.tensor.transpose(vt_ps[:], v_sb[:, t * D:(t + 1) * D], ident[:])
                vt_sb = spool.tile([128, 128], f32, tag="vt_sb")
                nc.vector.tensor_copy(out=vt_sb[:], in_=vt_ps[:])
                w_ps = pspool.tile([128, E * D], f32, tag="w_ps")
                nc.tensor.matmul(w_ps[:], lhsT=vt_sb[:], rhs=mt_sb[:], start=True, stop=True)

                # out[b] = sum_(e,d) agg*w + sum_d c*v
                prod = spool.tile([128, E * D], f32, tag="prod")
                nc.vector.tensor_mul(out=prod[:], in0=agg[:], in1=w_ps[:])
                r1 = spool.tile([128, 1], f32, tag="r1")
                nc.vector.reduce_sum(r1[:], prod[:], axis=mybir.AxisListType.X)
                cv = spool.tile([128, D], f32, tag="cv")
                nc.vector.tensor_mul(
                    out=cv[:], in0=v_sb[:, t * D:(t + 1) * D],
                    in1=c_bc[:],
                )
                r2 = spool.tile([128, 1], f32, tag="r2")
                nc.vector.reduce_sum(r2[:], cv[:], axis=mybir.AxisListType.X)
                nc.vector.tensor_add(out=res_sb[:, t:t + 1], in0=r1[:], in1=r2[:])

            sig_sb = cpool.tile([128, NT], f32)
            nc.scalar.activation(sig_sb[:], res_sb[:], AF.Sigmoid)
            nc.sync.dma_start(out=out_d[:], in_=sig_sb[:])

    nc.compile()
    return nc


def _get_nc(reps=1, nqueues=1, tmode="f32", gbufs=2):
    key = ("nc", reps, nqueues, tmode, gbufs)
    if key not in _CACHE:
        _CACHE[key] = _build_nc(reps, nqueues, tmode, gbufs)
    return _CACHE[key]


def prep_inputs(neighbors, end_node, path, emb_start, emb_end, emb_path, W1, b1, W2, b2,
                tmode="f32"):
    """Host-side shard + layout prep. Returns list of per-core input maps.

    tmode: table dtype for the two 1M-row embedding tables.
      f32 - as-is; f16 - cast; f8 - x16 pre-scale then e4m3 cast (the x16 and
      the fp8 product scale are folded into the on-chip M and c constants).
    """
    import ml_dtypes
    if tmode == "f8":
        edt, escale, pdt = ml_dtypes.float8_e4m3, 16.0, np.float16
    elif tmode in ("f16", "f16c"):
        edt, escale, pdt = np.float16, 1.0, np.float16
    else:
        edt, escale, pdt = np.float32, 1.0, np.float32
    neighbors = np.asarray(neighbors).astype(np.int32)
    end_node = np.asarray(end_node).astype(np.int32)
    path = np.asarray(path).astype(np.int32)
    emb_start = np.ascontiguousarray((np.asarray(emb_start, np.float32) * escale).astype(edt))
    emb_end = np.ascontiguousarray((np.asarray(emb_end, np.float32) * escale).astype(edt))
    emb_path = np.ascontiguousarray(np.asarray(emb_path, np.float32).astype(pdt))
    W1 = np.ascontiguousarray(np.asarray(W1, dtype=np.float32))
    b1 = np.ascontiguousarray(np.asarray(b1, dtype=np.float32)).reshape(D, 1)
    W2 = np.ascontiguousarray(np.asarray(W2, dtype=np.float32))
    b2 = np.ascontiguousarray(np.asarray(b2, dtype=np.float32)).reshape(1, D)

    in_maps = []
    for c in range(NCORES):
        lo = c * BL
        nb = neighbors[lo:lo + BL].reshape(NT, 128, E * K)      # [t, p, 64]
        idx_nei = np.ascontiguousarray(nb.transpose(1, 0, 2).reshape(128, NT * ROWS_PER_TILE))
        idx_end = np.ascontiguousarray(end_node[lo:lo + BL].reshape(NT, 128).T)
        idx_path = np.ascontiguousarray(path[lo:lo + BL].reshape(NT, 128).T)
        in_maps.append({
            "emb_s": emb_start, "emb_e": emb_end, "emb_p": emb_path,
            "idx_nei": idx_nei, "idx_end": idx_end, "idx_path": idx_path,
            "w1": W1, "b1": b1, "w2": W2, "b2": b2,
        })
    return in_maps


def assemble_output(results):
    """results: list of per-core dicts with 'out' [128, NT] -> full [B] f32."""
    outs = []
    for c in range(NCORES):
        r = results[c]["out"]           # [128, NT], out[p, t] = batch lo + t*128 + p
        outs.append(np.ascontiguousarray(r.T).reshape(BL))
    return np.concatenate(outs).astype(np.float32)


# Best measured config (see sweeps): table dtype / SWDGE queues / gather depth.
BEST = {"nqueues": 1, "tmode": "f32", "gbufs": 2}


def kernel(neighbors, end_node, path, emb_start, emb_end, emb_path, W1, b1, W2, b2):
    from concourse.bass_utils import run_bass_kernel_spmd

    nc = _get_nc(1, BEST["nqueues"], BEST["tmode"], BEST["gbufs"])
    in_maps = prep_inputs(neighbors, end_node, path, emb_start, emb_end,
                          emb_path, W1, b1, W2, b2, tmode=BEST["tmode"])
    res = run_bass_kernel_spmd(nc, in_maps, core_ids=list(range(NCORES)))
    return assemble_output(res.results)



# revision 8
# speedup vs baseline: 1.0865x; 1.0865x over previous
"""HIN2vec forward kernel for 8 Trainium2 NeuronCores.

Math (per batch element b):
    agg[e]  = mean_k emb_start[neighbors[b,e,k]]          # [E, D]
    h[e]    = agg[e] @ W1 + b1                            # [E, D]
    s       = concat_e(h) @ W2 + b2                       # [D]
    v       = emb_end[end[b]] * sigmoid(emb_path[path[b]])# [D]
    out[b]  = sigmoid(s . v)

Rewritten to make the device work gather-dominated:
    s . v = sum_e agg_sum[e] . (v @ M_e^T)/16  +  c . v
      where M_e = W1 @ W2_e  (W2_e = W2[e*D:(e+1)*D]),
            c   = b2 + b1 @ (sum_e W2_e),
            agg_sum[e] = sum_k emb_start[neighbors[b,e,k]]  (no /K; folded into M).

So the 1M-row table gathers (the memory roofline) feed only a K-sum (DVE
tree adds) and one elementwise multiply + reduction; the Linear layers act
on v via tiny PE matmuls (v is 32x smaller than the gathered data).

Sharding: pure data parallel over the batch; tables + weights replicated.

Table dtype: f32 (tmode="f32"). The random 512B-row gather is row-rate
bound on the SDMA engines, not byte-bound: on HW, fp16 tables timed 171us
vs f32 198.5us (only ~14%) and fp8 176.9us, but every sub-512B-row variant
produced NaN on real hardware (CoreSim passed) — including one that casts
the fp16 gather straight to f32 before any compute, isolating the fault to
the fp16 indirect_dma_start itself — so f32 is kept. Spreading the
indirect DMAs over 2-4 SWDGE queues or deepening the gather prefetch past
2 buffers both measured slightly WORSE (172-200us at f32), hence
nqueues=1, gbufs=2.
"""

import numpy as np

B, E, K, D = 8192, 4, 16, 128
N, PPATH = 1_000_000, 64
NCORES = 8
BL = B // NCORES          # 1024 batch elements per core
NT = BL // 128            # 8 b-tiles of 128 per core
ROWS_PER_TILE = E * K     # 64 gathered rows per batch element

_CACHE = {}


def _build_nc(reps=1, nqueues=1, tmode="f32", gbufs=2):
    import concourse.bacc as bacc
    import concourse.bass as bass
    import concourse.mybir as mybir
    import concourse.tile as tile
    from concourse.masks import make_identity

    f32 = mybir.dt.float32
    f16 = mybir.dt.float16
    i32 = mybir.dt.int32
    tdt = {"f32": f32, "f16": mybir.dt.float16, "f16c": mybir.dt.float16,
           "f8": mybir.dt.float8e4, "f32v": f32}[tmode]
    view32 = tmode == "f32v"  # table bytes are fp16, gathered as f32 pairs
    row_elems = 64 if view32 else 128   # gathered elems per table row
    pdt = mybir.dt.float16 if tmode == "f8" else (f32 if view32 else tdt)
    cast16 = tmode == "f16c"  # gather fp16, cast to f32 before any compute
    AF = mybir.ActivationFunctionType

    nc = bacc.Bacc("TRN2", target_bir_lowering=False, debug=False,
                   num_swdge_queues=nqueues)

    def indirect_on_queue(qi, **kw):
        bi = nc.gpsimd.indirect_dma_start(**kw)
        if qi % nqueues:
            bi.ins.queue = f"qPoolDynamic{qi % nqueues}"
        return bi

    emb_s = nc.dram_tensor("emb_s", [N, row_elems], tdt, kind="ExternalInput")
    emb_e = nc.dram_tensor("emb_e", [N, row_elems], tdt, kind="ExternalInput")
    emb_p = nc.dram_tensor("emb_p", [PPATH, D], pdt, kind="ExternalInput")
    idx_nei = nc.dram_tensor("idx_nei", [128, NT * ROWS_PER_TILE], i32, kind="ExternalInput")
    idx_end = nc.dram_tensor("idx_end", [128, NT], i32, kind="ExternalInput")
    idx_path = nc.dram_tensor("idx_path", [128, NT], i32, kind="ExternalInput")
    w1_d = nc.dram_tensor("w1", [D, D], f32, kind="ExternalInput")
    b1_d = nc.dram_tensor("b1", [D, 1], f32, kind="ExternalInput")
    w2_d = nc.dram_tensor("w2", [E * D, D], f32, kind="ExternalInput")
    b2_d = nc.dram_tensor("b2", [1, D], f32, kind="ExternalInput")
    out_d = nc.dram_tensor("out", [128, NT], f32, kind="ExternalOutput")

    with tile.TileContext(nc) as tc:
        with (
            tc.tile_pool(name="const", bufs=1) as cpool,
            tc.tile_pool(name="gather", bufs=gbufs) as gpool,
            tc.tile_pool(name="tree", bufs=2) as tpool,
            tc.tile_pool(name="agg", bufs=2) as apool,
            tc.tile_pool(name="small", bufs=2) as spool,
            tc.tile_pool(name="psum_pre", bufs=1, space="PSUM") as ppre,
            tc.tile_pool(name="psum", bufs=2, space="PSUM") as pspool,
        ):
            # ---- constant loads ----
            idxn_sb = cpool.tile([128, NT * ROWS_PER_TILE], i32)
            nc.sync.dma_start(out=idxn_sb[:], in_=idx_nei[:])
            idxe_sb = cpool.tile([128, NT], i32)
            nc.sync.dma_start(out=idxe_sb[:], in_=idx_end[:])
            idxp_sb = cpool.tile([128, NT], i32)
            nc.sync.dma_start(out=idxp_sb[:], in_=idx_path[:])

            w1_sb = cpool.tile([128, D], f32)
            nc.sync.dma_start(out=w1_sb[:], in_=w1_d[:])
            w2_sb = cpool.tile([128, E * D], f32)
            for e in range(E):
                nc.sync.dma_start(
                    out=w2_sb[:, e * D:(e + 1) * D],
                    in_=w2_d[e * D:(e + 1) * D, :],
                )
            b1_sb = cpool.tile([128, 1], f32)
            nc.sync.dma_start(out=b1_sb[:], in_=b1_d[:])
            b2_sb = cpool.tile([1, D], f32)
            nc.sync.dma_start(out=b2_sb[:], in_=b2_d[:])

            ident = cpool.tile([128, 128], f32)
            make_identity(nc, ident[:])

            # ---- prelude: MT_all[f, e*D+d] = (W1^T/16 . W2_e)[f,d]^T ... i.e.
            # MT_e[f, d] = sum_x W2_e[x, f] * W1T[x, d] / 16  = (M_e/16)^T
            w1t_ps = ppre.tile([128, 128], f32)
            nc.tensor.transpose(w1t_ps[:], w1_sb[:], ident[:])
            w1t_sb = cpool.tile([128, 128], f32)
            w1t_scale = 1.0 / (K * 256.0) if tmode == "f8" else 1.0 / K
            nc.scalar.mul(w1t_sb[:], w1t_ps[:], w1t_scale)
            mt_sb = cpool.tile([128, E * D], f32)
            for e in range(E):
                mt_ps = ppre.tile([128, 128], f32, tag="mt_ps")
                nc.tensor.matmul(
                    mt_ps[:],
                    lhsT=w2_sb[:, e * D:(e + 1) * D],
                    rhs=w1t_sb[:],
                    start=True, stop=True,
                )
                nc.vector.tensor_copy(out=mt_sb[:, e * D:(e + 1) * D], in_=mt_ps[:])

            # c = b2 + b1 @ (sum_e W2_e)   -> [1, D]
            wsum_sb = cpool.tile([128, D], f32)
            nc.vector.tensor_add(out=wsum_sb[:], in0=w2_sb[:, 0:D], in1=w2_sb[:, D:2 * D])
            nc.vector.tensor_add(out=wsum_sb[:], in0=wsum_sb[:], in1=w2_sb[:, 2 * D:3 * D])
            nc.vector.tensor_add(out=wsum_sb[:], in0=wsum_sb[:], in1=w2_sb[:, 3 * D:4 * D])
            c_ps = ppre.tile([1, D], f32)
            nc.tensor.matmul(c_ps[:], lhsT=b1_sb[:], rhs=wsum_sb[:], start=True, stop=True)
            c_sb = cpool.tile([1, D], f32)
            nc.vector.tensor_add(out=c_sb[:], in0=c_ps[:], in1=b2_sb[:])
            if tmode == "f8":
                nc.scalar.mul(c_sb[:], c_sb[:], 1.0 / 16.0)
            c_bc = cpool.tile([128, D], f32)
            nc.gpsimd.partition_broadcast(c_bc[:], c_sb[:])

            # ---- v = emb_end[end] * sigmoid(emb_path[path]) : [128, NT*D] ----
            ve_sb = cpool.tile([128, NT * row_elems], tdt)
            indirect_on_queue(1,
                out=ve_sb[:], out_offset=None,
                in_=emb_e[:],
                in_offset=bass.IndirectOffsetOnAxis(ap=idxe_sb[:], axis=0),
            )
            vp_sb = cpool.tile([128, NT * D], pdt)
            indirect_on_queue(2,
                out=vp_sb[:], out_offset=None,
                in_=emb_p[:],
                in_offset=bass.IndirectOffsetOnAxis(ap=idxp_sb[:], axis=0),
            )
            if cast16:
                ve32 = cpool.tile([128, NT * D], f32)
                nc.vector.tensor_copy(out=ve32[:], in_=ve_sb[:])
                vp32 = cpool.tile([128, NT * D], f32)
                nc.vector.tensor_copy(out=vp32[:], in_=vp_sb[:])
            elif view32:
                ve32 = cpool.tile([128, NT * D], f32)
                nc.vector.tensor_copy(out=ve32[:], in_=ve_sb[:].bitcast(f16))
                vp32 = vp_sb
            else:
                ve32, vp32 = ve_sb, vp_sb
            v_sb = cpool.tile([128, NT * D], f32)
            nc.scalar.activation(v_sb[:], vp32[:], AF.Sigmoid)
            nc.vector.tensor_mul(out=v_sb[:], in0=v_sb[:], in1=ve32[:])

            res_sb = cpool.tile([128, NT], f32)

            # ---- main loop over b-tiles ----
            for t in range(NT * reps):
                t = t % NT
                g = gpool.tile([128, ROWS_PER_TILE * row_elems], tdt, tag="g")
                indirect_on_queue(t,
                    out=g[:], out_offset=None,
                    in_=emb_s[:],
                    in_offset=bass.IndirectOffsetOnAxis(
                        ap=idxn_sb[:, t * ROWS_PER_TILE:(t + 1) * ROWS_PER_TILE],
                        axis=0,
                    ),
                )
                if cast16:
                    g32 = gpool.tile([128, ROWS_PER_TILE * D], f32, tag="g32")
                    nc.vector.tensor_copy(out=g32[:], in_=g[:])
                    g = g32

                # sum over k (tree): [128, e, k, d] -> [128, e, d]
                if view32:
                    gv = g[:].bitcast(f16).rearrange(
                        "p (e k d) -> p e k d", e=E, k=K, d=D)
                else:
                    gv = g[:].rearrange("p (e k d) -> p e k d", e=E, k=K, d=D)
                t1 = tpool.tile([128, E * 8 * D], f32, tag="t1")
                t1v = t1[:].rearrange("p (e k d) -> p e k d", e=E, k=8, d=D)
                nc.vector.tensor_add(out=t1v, in0=gv[:, :, 0:K:2, :], in1=gv[:, :, 1:K:2, :])
                t2 = tpool.tile([128, E * 4 * D], f32, tag="t2")
                t2v = t2[:].rearrange("p (e k d) -> p e k d", e=E, k=4, d=D)
                nc.vector.tensor_add(out=t2v, in0=t1v[:, :, 0:8:2, :], in1=t1v[:, :, 1:8:2, :])
                t3 = tpool.tile([128, E * 2 * D], f32, tag="t3")
                t3v = t3[:].rearrange("p (e k d) -> p e k d", e=E, k=2, d=D)
                nc.vector.tensor_add(out=t3v, in0=t2v[:, :, 0:4:2, :], in1=t2v[:, :, 1:4:2, :])
                agg = apool.tile([128, E * D], f32, tag="agg")
                aggv = agg[:].rearrange("p (e k d) -> p e k d", e=E, k=1, d=D)
                nc.vector.tensor_add(out=aggv, in0=t3v[:, :, 0:2:2, :], in1=t3v[:, :, 1:2:2, :])

                # w[b, (e,d)] = sum_f v[b, f] * MT_all[f, (e,d)]
                vt_ps = pspool.tile([128, 128], f32, tag="vt_ps")
                nc.tensor.transpose(vt_ps[:], v_sb[:, t * D:(t + 1) * D], ident[:])
                vt_sb = spool.tile([128, 128], f32, tag="vt_sb")
                nc.vector.tensor_copy(out=vt_sb[:], in_=vt_ps[:])
                w_ps = pspool.tile([128, E * D], f32, tag="w_ps")
                nc.tensor.matmul(w_ps[:], lhsT=vt_sb[:], rhs=mt_sb[:], start=True, stop=True)

                # out[b] = sum_(e,d) agg*w + sum_d c*v
                prod = spool.tile([128, E * D], f32, tag="prod")
                nc.vector.tensor_mul(out=prod[:], in0=agg[:], in1=w_ps[:])
                r1 = spool.tile([128, 1], f32, tag="r1")
                nc.vector.reduce_sum(r1[:], prod[:], axis=mybir.AxisListType.X)
                cv = spool.tile([128, D], f32, tag="cv")
                nc.vector.tensor_mul(
                    out=cv[:], in0=v_sb[:, t * D:(t + 1) * D],
                    in1=c_bc[:],
                )
                r2 = spool.tile([128, 1], f32, tag="r2")
                nc.vector.reduce_sum(r2[:], cv[:], axis=mybir.AxisListType.X)
                nc.vector.tensor_add(out=res_sb[:, t:t + 1], in0=r1[:], in1=r2[:])

            sig_sb = cpool.tile([128, NT], f32)
            nc.scalar.activation(sig_sb[:], res_sb[:], AF.Sigmoid)
            nc.sync.dma_start(out=out_d[:], in_=sig_sb[:])

    nc.compile()
    return nc


def _get_nc(reps=1, nqueues=1, tmode="f32", gbufs=2):
    key = ("nc", reps, nqueues, tmode, gbufs)
    if key not in _CACHE:
        _CACHE[key] = _build_nc(reps, nqueues, tmode, gbufs)
    return _CACHE[key]


def prep_inputs(neighbors, end_node, path, emb_start, emb_end, emb_path, W1, b1, W2, b2,
                tmode="f32"):
    """Host-side shard + layout prep. Returns list of per-core input maps.

    tmode: table dtype for the two 1M-row embedding tables.
      f32 - as-is; f16 - cast; f8 - x16 pre-scale then e4m3 cast (the x16 and
      the fp8 product scale are folded into the on-chip M and c constants).
    """
    import ml_dtypes
    if tmode == "f8":
        edt, escale, pdt = ml_dtypes.float8_e4m3, 16.0, np.float16
    elif tmode in ("f16", "f16c"):
        edt, escale, pdt = np.float16, 1.0, np.float16
    elif tmode == "f32v":
        edt, escale, pdt = np.float16, 1.0, np.float32
    else:
        edt, escale, pdt = np.float32, 1.0, np.float32
    neighbors = np.asarray(neighbors).astype(np.int32)
    end_node = np.asarray(end_node).astype(np.int32)
    path = np.asarray(path).astype(np.int32)
    emb_start = np.ascontiguousarray((np.asarray(emb_start, np.float32) * escale).astype(edt))
    emb_end = np.ascontiguousarray((np.asarray(emb_end, np.float32) * escale).astype(edt))
    if tmode == "f32v":  # fp16 bytes reinterpreted as f32 pairs -> [N, 64]
        emb_start = emb_start.view(np.float32)
        emb_end = emb_end.view(np.float32)
    emb_path = np.ascontiguousarray(np.asarray(emb_path, np.float32).astype(pdt))
    W1 = np.ascontiguousarray(np.asarray(W1, dtype=np.float32))
    b1 = np.ascontiguousarray(np.asarray(b1, dtype=np.float32)).reshape(D, 1)
    W2 = np.ascontiguousarray(np.asarray(W2, dtype=np.float32))
    b2 = np.ascontiguousarray(np.asarray(b2, dtype=np.float32)).reshape(1, D)

    in_maps = []
    for c in range(NCORES):
        lo = c * BL
        nb = neighbors[lo:lo + BL].reshape(NT, 128, E * K)      # [t, p, 64]
        idx_nei = np.ascontiguousarray(nb.transpose(1, 0, 2).reshape(128, NT * ROWS_PER_TILE))
        idx_end = np.ascontiguousarray(end_node[lo:lo + BL].reshape(NT, 128).T)
        idx_path = np.ascontiguousarray(path[lo:lo + BL].reshape(NT, 128).T)
        in_maps.append({
            "emb_s": emb_start, "emb_e": emb_end, "emb_p": emb_path,
            "idx_nei": idx_nei, "idx_end": idx_end, "idx_path": idx_path,
            "w1": W1, "b1": b1, "w2": W2, "b2": b2,
        })
    return in_maps


def assemble_output(results):
    """results: list of per-core dicts with 'out' [128, NT] -> full [B] f32."""
    outs = []
    for c in range(NCORES):
        r = results[c]["out"]           # [128, NT], out[p, t] = batch lo + t*128 + p
        outs.append(np.ascontiguousarray(r.T).reshape(BL))
    return np.concatenate(outs).astype(np.float32)


# Best measured config (see sweeps): table dtype / SWDGE queues / gather depth.
BEST = {"nqueues": 1, "tmode": "f32v", "gbufs": 2}


def kernel(neighbors, end_node, path, emb_start, emb_end, emb_path, W1, b1, W2, b2):
    from concourse.bass_utils import run_bass_kernel_spmd

    nc = _get_nc(1, BEST["nqueues"], BEST["tmode"], BEST["gbufs"])
    in_maps = prep_inputs(neighbors, end_node, path, emb_start, emb_end,
                          emb_path, W1, b1, W2, b2, tmode=BEST["tmode"])
    res = run_bass_kernel_spmd(nc, in_maps, core_ids=list(range(NCORES)))
    return assemble_output(res.results)

